# revision 12
# baseline (speedup 1.0000x reference)
"""Trainium2 Bass kernel for nn_AutoEncoder (topk SAE), 8-core batch-parallel.

reference:
    project = (embed - bias) @ enc_weight.T          # [B, F] fp32
    weights, feats = top_k(project, 64)
    total = bincount(feats); new_last_usage = (last_usage + B) * (total == 0)
    recon = sum_k weights * lookup[feats] + bias; embed1 = l2normalize(recon)

Strategy:
  - Data-parallel over batch: 512 rows/core, no collectives.
  - Encoder at fp32 precision via 3-term fp32r split (HW fp32r keeps ~11
    mantissa bits at bf16 speed): Xh*Wh + Xl*Wh + Xh*Wl with host-side
    13-bit-drop split of W (clean values are fixed points of HW rounding).
  - Exact per-row top-64 threshold tau via per-128-chunk top-8 (nc.vector.max)
    + 8-round merge w/ match_replace; per-row flags certify exactness
    (count==64, no chunk with >=8 values >= tau, tau>0).
  - Decode: masked = project * (project >= tau) in fp16, PE-transposed blocks
    against fp16 lookup, accumulated over F in PSUM chains + SBUF adds.
  - Feature-usage: per-feature count of selection via PE ones-matmul over the
    0/1 mask; host combines cores and applies the (total==0) formula.
"""

import numpy as np
import concourse.bass as bass
import concourse.mybir as mybir
from concourse.tile import TileContext
from concourse.bass_utils import run_bass_kernel_spmd

FP32 = mybir.dt.float32
FP32R = mybir.dt.float32r
FP16 = mybir.dt.float16
FP8 = mybir.dt.float8e4
BF16 = mybir.dt.bfloat16

B, E, F, K = 4096, 2048, 32768, 64
NCORES = 8
BP = B // NCORES            # 512 rows per core
NBT = BP // 128             # 4 b-tiles
NE = E // 128               # 16 contraction chunks
NFC = F // 512              # 64 encoder f-chunks
NFB = F // 128              # 256 decode f-blocks
NEC = E // 512              # 4 decode output chunks
DGRP = 8                    # f-blocks per decode group (1024 f)
NG = NFB // DGRP            # 32 groups

LAST_EXEC_NS = None


def _split_multiwait(nc):
    """This walrus build allows ONE sync-wait per instruction; Tile emits
    several. Insert single-wait NOPs before such instructions (same engine,
    program order preserves semantics)."""
    n = 0
    for f in nc.m.functions:
        for blk in f.blocks:
            newlist = []
            for ins in blk.instructions:
                si = ins.sync_info
                waits = list(si.on_wait) if si and si.on_wait else []
                if len(waits) > 1:
                    for w in waits[:-1]:
                        n += 1
                        nop = mybir.InstNoOp(name=f"I-mwfix-{n}", ins=[], outs=[])
                        nop.engine = ins.engine
                        nop.sync_info = mybir.SyncInfo(on_wait=[w], on_update=[])
                        newlist.append(nop)
                    ins.sync_info = mybir.SyncInfo(
                        on_wait=[waits[-1]], on_update=list(si.on_update or []))
                newlist.append(ins)
            blk.instructions = newlist
    return n


def build_nc():
    nc = bass.Bass()
    AL = mybir.AluOpType
    AX = mybir.AxisListType

    xt_d = nc.dram_tensor("xt", [E, BP], FP32, kind="ExternalInput")       # embed^T slice
    bias_d = nc.dram_tensor("biasv", [E, 1], FP32, kind="ExternalInput")
    biasr_d = nc.dram_tensor("biasr", [1, E], FP32, kind="ExternalInput")
    wth_d = nc.dram_tensor("wth", [E, F], FP32R, kind="ExternalInput")     # W^T hi (13-bit clean)
    whf8_d = nc.dram_tensor("whf8", [E, F], FP8, kind="ExternalInput")     # fp8(W^T hi)
    wlf8_d = nc.dram_tensor("wlf8", [E, F], FP8, kind="ExternalInput")     # fp8(W^T lo * 2^12)
    lk_d = nc.dram_tensor("lk", [F, E], FP16, kind="ExternalInput")        # lookup fp16
    id16_d = nc.dram_tensor("id16", [128, 128], FP16, kind="ExternalInput")

    out_d = nc.dram_tensor("out_embed", [BP, E], FP32, kind="ExternalOutput")
    tot_d = nc.dram_tensor("out_totals", [1, F], FP32, kind="ExternalOutput")
    flg_d = nc.dram_tensor("out_flags", [128, 3 * NBT], FP32, kind="ExternalOutput")

    with TileContext(nc) as tc:
        with tc.tile_pool(name="singles", bufs=1) as sing, \
             tc.tile_pool(name="dram", bufs=1, space="DRAM") as dpool:
            proj_d = dpool.tile([NBT * 128, F], FP32)

            id16 = sing.tile([128, 128], FP16)
            nc.gpsimd.dma_start(id16[:], id16_d[:])
            ones1 = sing.tile([1, 128], FP32)
            nc.vector.memset(ones1[:], 1.0)
            ones128 = sing.tile([128, 1], BF16)
            nc.vector.memset(ones128[:], 1.0)
            biasrow = sing.tile([1, E], FP32)
            nc.gpsimd.dma_start(biasrow[:], biasr_d[:])
            flags = sing.tile([128, 3 * NBT], FP32)   # per bt: [count, danger, tau]
            taus = sing.tile([128, NBT], FP32)

            # ---------- Phase A: encode + chunk-top8 + proj -> DRAM ----------
            with tc.tile_pool(name="phAB", bufs=1) as pab:
                m8s = pab.tile([128, NBT * 2048], FP32)   # per-bt chunk-top8s
                with tc.tile_pool(name="phA", bufs=1) as pa, \
                     tc.tile_pool(name="wstream", bufs=3) as ws, \
                     tc.tile_pool(name="chb", bufs=3) as chb, \
                     tc.tile_pool(name="psA", bufs=2, space="PSUM") as psA:
                    xhs_all = pa.tile([128, NE * BP], FP32R)   # Xh * 2^12
                    xh8_all = pa.tile([128, NE * BP], FP8)
                    xl8_all = pa.tile([128, NE * BP], FP8)
                    for e in range(NE):
                        xa = pa.tile([128, BP], FP32, name=f"xa{e}", tag="xa", bufs=2)
                        nc.sync.dma_start(xa[:], xt_d[e * 128:(e + 1) * 128, :])
                        bcol = pa.tile([128, 1], FP32, name=f"bc{e}", tag="bc", bufs=2)
                        nc.sync.dma_start(bcol[:], bias_d[e * 128:(e + 1) * 128, :])
                        nc.vector.tensor_scalar(xa[:], xa[:], bcol[:], None,
                                                op0=AL.subtract)
                        xh = pa.tile([128, BP], FP32R, name=f"xh{e}", tag="xh", bufs=2)
                        nc.vector.tensor_copy(xh[:], xa[:])
                        xl = pa.tile([128, BP], FP32, name=f"xl{e}", tag="xl", bufs=2)
                        nc.vector.tensor_tensor(out=xl[:], in0=xa[:],
                                                in1=xh[:].bitcast(FP32),
                                                op=AL.subtract)
                        nc.vector.tensor_copy(xh8_all[:, e * BP:(e + 1) * BP],
                                              xh[:].bitcast(FP32))
                        nc.vector.tensor_scalar(xl8_all[:, e * BP:(e + 1) * BP],
                                                xl[:], 4096.0, None, op0=AL.mult)
                        nc.vector.tensor_scalar(xhs_all[:, e * BP:(e + 1) * BP],
                                                xh[:].bitcast(FP32), 4096.0, None,
                                                op0=AL.mult)

                    NSC = NE // 2  # 8 superchunks of 256 e
                    xh8r = xh8_all.rearrange("p (e b) -> p e b", b=BP)
                    xl8r = xl8_all.rearrange("p (e b) -> p e b", b=BP)
                    for fc in range(NFC):
                        pss = [psA.tile([128, 512], FP32, name=f"psA{bt}_{fc}",
                                        tag=f"psA{bt}") for bt in range(NBT)]
                        for e in range(NE):
                            wh = ws.tile([128, 512], FP32R, name=f"wh{fc}_{e}", tag="wh")
                            nc.sync.dma_start(wh[:], wth_d[e * 128:(e + 1) * 128,
                                                           fc * 512:(fc + 1) * 512])
                            for bt in range(NBT):
                                xh_b = xhs_all[:, e * BP + bt * 128:e * BP + (bt + 1) * 128]
                                nc.tensor.matmul(pss[bt][:], xh_b, wh[:],
                                                 start=(e == 0), stop=False)
                        for sc in range(NSC):
                            w8h = ws.tile([128, 1024], FP8, name=f"w8h{fc}_{sc}",
                                          tag="w8h")
                            nc.sync.dma_start(
                                w8h[:].rearrange("p (i n) -> p i n", i=2),
                                whf8_d[sc * 256:(sc + 1) * 256,
                                       fc * 512:(fc + 1) * 512].rearrange(
                                           "(i p) n -> p i n", i=2))
                            w8l = ws.tile([128, 1024], FP8, name=f"w8l{fc}_{sc}",
                                          tag="w8l")
                            nc.sync.dma_start(
                                w8l[:].rearrange("p (i n) -> p i n", i=2),
                                wlf8_d[sc * 256:(sc + 1) * 256,
                                       fc * 512:(fc + 1) * 512].rearrange(
                                           "(i p) n -> p i n", i=2))
                            for bt in range(NBT):
                                xl8_b = xl8r[:, 2 * sc:2 * sc + 2,
                                             bt * 128:(bt + 1) * 128]
                                xh8_b = xh8r[:, 2 * sc:2 * sc + 2,
                                             bt * 128:(bt + 1) * 128]
                                nc.tensor.matmul(
                                    pss[bt][:], xl8_b,
                                    w8h[:].rearrange("p (i n) -> p i n", i=2),
                                    start=False, stop=False,
                                    perf_mode=mybir.MatmulPerfMode.DoubleRow)
                                nc.tensor.matmul(
                                    pss[bt][:], xh8_b,
                                    w8l[:].rearrange("p (i n) -> p i n", i=2),
                                    start=False, stop=(sc == NSC - 1),
                                    perf_mode=mybir.MatmulPerfMode.DoubleRow)
                        for bt in range(NBT):
                            cb = chb.tile([128, 512], FP32, name=f"cb{bt}_{fc}",
                                          tag=f"cb{bt}")
                            nc.scalar.mul(out=cb[:], in_=pss[bt][:],
                                          mul=float(2.0 ** -12))
                            for s in range(4):
                                slot = m8s[:, bt * 2048 + fc * 32 + s * 8:
                                           bt * 2048 + fc * 32 + (s + 1) * 8]
                                nc.vector.max(out=slot, in_=cb[:, s * 128:(s + 1) * 128])
                            nc.sync.dma_start(
                                proj_d[bt * 128:(bt + 1) * 128,
                                       fc * 512:(fc + 1) * 512], cb[:])

                # ---------- Phase B: merge -> tau per bt ----------
                with tc.tile_pool(name="phB", bufs=1) as pb:
                    for bt in range(NBT):
                        m8 = m8s[:, bt * 2048:(bt + 1) * 2048]
                        eighth = pb.tile([128, 256], FP32, name=f"e8{bt}", tag="e8",
                                         bufs=2)
                        nc.vector.tensor_copy(
                            eighth[:], m8.rearrange("p (c k) -> p c k", k=8)[:, :, 7])
                        t8 = pb.tile([128, 8], FP32, name=f"t8{bt}", tag="t8", bufs=2)
                        for r in range(8):
                            nc.vector.max(out=t8[:], in_=m8)
                            if r < 7:
                                nc.vector.match_replace(out=m8, in_to_replace=t8[:],
                                                        in_values=m8, imm_value=-1e30)
                        nc.vector.tensor_copy(taus[:, bt:bt + 1], t8[:, 7:8])
                        dm = pb.tile([128, 256], FP32, name=f"dm{bt}", tag="dm", bufs=2)
                        nc.vector.tensor_scalar(dm[:], eighth[:], taus[:, bt:bt + 1],
                                                None, op0=AL.is_ge)
                        nc.vector.tensor_reduce(out=flags[:, 3 * bt + 1:3 * bt + 2],
                                                in_=dm[:], axis=AX.X, op=AL.add)
                        nc.vector.tensor_copy(flags[:, 3 * bt + 2:3 * bt + 3],
                                              taus[:, bt:bt + 1])

            # ---------- Phase D: mask + transpose + decode + totals ----------
            GW = DGRP * 128  # 1024 f per group
            with tc.tile_pool(name="phD", bufs=1) as pd_, \
                 tc.tile_pool(name="lkp", bufs=2) as lkp, \
                 tc.tile_pool(name="mtp", bufs=2) as mtp, \
                 tc.tile_pool(name="psD", bufs=2, space="PSUM") as psD:
                acc = pd_.tile([128, NBT * E], FP32)
                nc.vector.memset(acc[:], 0.0)
                counts = pd_.tile([128, NBT], FP32)
                nc.vector.memset(counts[:], 0.0)
                biasf = pd_.tile([128, E], FP32)
                for ec in range(NEC):
                    psb = psD.tile([128, 512], FP32, name=f"psb{ec}", tag="psacc")
                    nc.tensor.matmul(psb[:], ones1[:],
                                     biasrow[:, ec * 512:(ec + 1) * 512],
                                     start=True, stop=True)
                    nc.vector.tensor_copy(biasf[:, ec * 512:(ec + 1) * 512], psb[:])

                for g in range(NG):
                    f0 = g * GW
                    lks = []
                    for j in range(DGRP):
                        fb = g * DGRP + j
                        lk = lkp.tile([128, E], FP16, name=f"lk{j}_{g}", tag=f"lk{j}",
                                      bufs=1)
                        nc.sync.dma_start(lk[:], lk_d[fb * 128:(fb + 1) * 128, :])
                        lks.append(lk)
                    mts = {}
                    mbs = {}
                    for bt in range(NBT):
                        pj = pd_.tile([128, GW], FP32, name=f"pj{bt}_{g}",
                                      tag=f"pj{bt}", bufs=2)
                        nc.sync.dma_start(pj[:], proj_d[bt * 128:(bt + 1) * 128,
                                                        f0:f0 + GW])
                        mb = pd_.tile([128, GW], BF16, name=f"mbm{bt}_{g}",
                                      tag=f"mbm{bt}", bufs=2)
                        mbs[bt] = mb
                        nc.vector.tensor_scalar(mb[:], pj[:], taus[:, bt:bt + 1],
                                                None, op0=AL.is_ge)
                        m16 = pd_.tile([128, GW], FP16, name=f"m16{bt}_{g}",
                                       tag=f"m16{bt}", bufs=2)
                        nc.vector.tensor_tensor(out=m16[:], in0=pj[:], in1=mb[:],
                                                op=AL.mult)
                        ctmp = pd_.tile([128, 1], FP32, name=f"ct{bt}_{g}",
                                        tag=f"ct{bt}", bufs=2)
                        nc.vector.tensor_reduce(out=ctmp[:], in_=mb[:], axis=AX.X,
                                                op=AL.add)
                        nc.vector.tensor_tensor(out=counts[:, bt:bt + 1],
                                                in0=counts[:, bt:bt + 1],
                                                in1=ctmp[:], op=AL.add)
                        for j in range(DGRP):
                            pst = psD.tile([128, 128], FP16, name=f"pst{j}_{bt}_{g}",
                                           tag="pst")
                            nc.tensor.transpose(pst[:], m16[:, j * 128:(j + 1) * 128],
                                                id16[:])
                            mt = mtp.tile([128, 128], FP16, name=f"mt{j}_{bt}_{g}",
                                          tag=f"mt{j}_{bt}", bufs=1)
                            nc.vector.tensor_copy(mt[:], pst[:])
                            mts[(j, bt)] = mt
                    # totals: column-sums of mask via ones-matmul, chained over bt
                    for h in range(GW // 512):
                        pstot = psD.tile([1, 512], FP32, name=f"pt{h}_{g}", tag="ptot")
                        for bt in range(NBT):
                            nc.tensor.matmul(
                                pstot[:], ones128[:],
                                mbs[bt][:, h * 512:(h + 1) * 512],
                                start=(bt == 0), stop=(bt == NBT - 1))
                        tst = pd_.tile([1, 512], FP32, name=f"ts{h}_{g}",
                                       tag="tst", bufs=2)
                        nc.vector.tensor_copy(tst[:], pstot[:])
                        nc.sync.dma_start(
                            tot_d[:, f0 + h * 512:f0 + (h + 1) * 512], tst[:])
                    for bt in range(NBT):
                        for ec in range(NEC):
                            ps = psD.tile([128, 512], FP32, name=f"da{bt}_{ec}_{g}",
                                          tag="psacc")
                            for j in range(DGRP):
                                nc.tensor.matmul(ps[:], mts[(j, bt)][:],
                                                 lks[j][:, ec * 512:(ec + 1) * 512],
                                                 start=(j == 0), stop=(j == DGRP - 1))
                            a = acc[:, bt * E + ec * 512:bt * E + (ec + 1) * 512]
                            nc.vector.tensor_tensor(out=a, in0=a, in1=ps[:], op=AL.add)

                # ---------- Phase E: bias + normalize + outputs ----------

                for bt in range(NBT):
                    st = pd_.tile([128, E], FP32, name=f"st{bt}", tag="st", bufs=1)
                    nc.vector.tensor_tensor(out=st[:], in0=acc[:, bt * E:(bt + 1) * E],
                                            in1=biasf[:], op=AL.add)
                    sq = pd_.tile([128, E], FP32, name=f"sq{bt}", tag="sq", bufs=1)
                    nc.vector.tensor_tensor(out=sq[:], in0=st[:], in1=st[:], op=AL.mult)
                    ss = pd_.tile([128, 1], FP32, name=f"ss{bt}", tag="ss", bufs=2)
                    nc.vector.tensor_reduce(out=ss[:], in_=sq[:], axis=AX.X, op=AL.add)
                    nrm = pd_.tile([128, 1], FP32, name=f"nm{bt}", tag="nm", bufs=2)
                    nc.scalar.sqrt(out=nrm[:], in_=ss[:])
                    nc.vector.tensor_scalar_max(nrm[:], nrm[:], 1e-12)
                    rcp = pd_.tile([128, 1], FP32, name=f"rc{bt}", tag="rc", bufs=2)
                    nc.vector.reciprocal(out=rcp[:], in_=nrm[:])
                    nc.vector.tensor_scalar(st[:], st[:], rcp[:], None, op0=AL.mult)
                    nc.sync.dma_start(out_d[bt * 128:(bt + 1) * 128, :], st[:])
                    nc.vector.tensor_copy(flags[:, 3 * bt:3 * bt + 1],
                                          counts[:, bt:bt + 1])

                nc.sync.dma_start(flg_d[:], flags[:])

    return nc


def _drop_bits(v, drop=13):
    b = v.view(np.uint32).astype(np.uint64)
    r = ((b + (1 << (drop - 1))) >> drop << drop).astype(np.uint32)
    return r.view(np.float32)


_NC_CACHE = [None]


def kernel(embed, last_usage, bias, enc_weight, lookup):
    global LAST_EXEC_NS
    embed = np.ascontiguousarray(np.asarray(embed, dtype=np.float32))
    last_usage = np.asarray(last_usage, dtype=np.int32)
    bias = np.ascontiguousarray(np.asarray(bias, dtype=np.float32))
    enc_weight = np.ascontiguousarray(np.asarray(enc_weight, dtype=np.float32))
    lookup = np.ascontiguousarray(np.asarray(lookup, dtype=np.float32))

    import ml_dtypes
    wt = np.ascontiguousarray(enc_weight.T)            # [E, F]
    wth = _drop_bits(wt)
    wtl = (wt - wth).astype(np.float32)
    whf8 = wth.astype(ml_dtypes.float8_e4m3).view(np.uint8)
    wlf8 = (wtl * 4096.0).astype(ml_dtypes.float8_e4m3).view(np.uint8)
    lk16 = lookup.astype(np.float16)
    id16 = np.eye(128, dtype=np.float16)
    biasv = bias.reshape(E, 1).copy()
    biasr = bias.reshape(1, E).copy()

    in_maps = []
    for c in range(NCORES):
        xt = np.ascontiguousarray(embed[c * BP:(c + 1) * BP, :].T)
        in_maps.append({"xt": xt, "biasv": biasv, "biasr": biasr,
                        "wth": wth, "whf8": whf8, "wlf8": wlf8,
                        "lk": lk16, "id16": id16})

    if _NC_CACHE[0] is None:
        nc = build_nc()
        _split_multiwait(nc)
        _NC_CACHE[0] = nc
    nc = _NC_CACHE[0]

    res = run_bass_kernel_spmd(nc, in_maps, core_ids=list(range(NCORES)))
    LAST_EXEC_NS = res.exec_time_ns

    embed1 = np.concatenate([res.results[c]["out_embed"] for c in range(NCORES)],
                            axis=0)

    total = np.zeros(F, dtype=np.float64)
    for c in range(NCORES):
        total += res.results[c]["out_totals"].reshape(F).astype(np.float64)
    selected = total > 0.0
    new_last_usage = ((last_usage.astype(np.int64) + B) * (~selected)).astype(np.int32)

    for c in range(NCORES):
        fl = res.results[c]["out_flags"]
        for bt in range(NBT):
            cnt, dng, tau = fl[:, 3 * bt], fl[:, 3 * bt + 1], fl[:, 3 * bt + 2]
            if not np.all(cnt == 64.0):
                print(f"WARNING core{c} bt{bt}: counts != 64:",
                      np.unique(cnt[cnt != 64.0])[:8])
            if not np.all(dng == 0.0):
                print(f"WARNING core{c} bt{bt}: danger chunks present")
            if not np.all(tau > 0.0):
                print(f"WARNING core{c} bt{bt}: tau <= 0")

    return embed1, new_last_usage


# revision 13
# speedup vs baseline: 1.1914x; 1.1914x over previous
"""Trainium2 Bass kernel for nn_AutoEncoder (topk SAE), 8-core batch-parallel.

reference:
    project = (embed - bias) @ enc_weight.T          # [B, F] fp32
    weights, feats = top_k(project, 64)
    total = bincount(feats); new_last_usage = (last_usage + B) * (total == 0)
    recon = sum_k weights * lookup[feats] + bias; embed1 = l2normalize(recon)

Strategy:
  - Data-parallel over batch: 512 rows/core, no collectives.
  - Encoder at fp32 precision via 3-term fp32r split (HW fp32r keeps ~11
    mantissa bits at bf16 speed): Xh*Wh + Xl*Wh + Xh*Wl with host-side
    13-bit-drop split of W (clean values are fixed points of HW rounding).
  - Exact per-row top-64 threshold tau via per-128-chunk top-8 (nc.vector.max)
    + 8-round merge w/ match_replace; per-row flags certify exactness
    (count==64, no chunk with >=8 values >= tau, tau>0).
  - Decode: masked = project * (project >= tau) in fp16, PE-transposed blocks
    against fp16 lookup, accumulated over F in PSUM chains + SBUF adds.
  - Feature-usage: per-feature count of selection via PE ones-matmul over the
    0/1 mask; host combines cores and applies the (total==0) formula.
"""

import numpy as np
import concourse.bass as bass
import concourse.mybir as mybir
from concourse.tile import TileContext
from concourse.bass_utils import run_bass_kernel_spmd

FP32 = mybir.dt.float32
FP32R = mybir.dt.float32r
FP16 = mybir.dt.float16
FP8 = mybir.dt.float8e4
BF16 = mybir.dt.bfloat16

B, E, F, K = 4096, 2048, 32768, 64
NCORES = 8
BP = B // NCORES            # 512 rows per core
NBT = BP // 128             # 4 b-tiles
NE = E // 128               # 16 contraction chunks
NFC = F // 512              # 64 encoder f-chunks
NFB = F // 128              # 256 decode f-blocks
NEC = E // 512              # 4 decode output chunks
DGRP = 8                    # f-blocks per decode group (1024 f)
NG = NFB // DGRP            # 32 groups

LAST_EXEC_NS = None


def _split_multiwait(nc):
    """This walrus build allows ONE sync-wait per instruction; Tile emits
    several. Insert single-wait NOPs before such instructions (same engine,
    program order preserves semantics)."""
    n = 0
    for f in nc.m.functions:
        for blk in f.blocks:
            newlist = []
            for ins in blk.instructions:
                si = ins.sync_info
                waits = list(si.on_wait) if si and si.on_wait else []
                if len(waits) > 1:
                    for w in waits[:-1]:
                        n += 1
                        nop = mybir.InstNoOp(name=f"I-mwfix-{n}", ins=[], outs=[])
                        nop.engine = ins.engine
                        nop.sync_info = mybir.SyncInfo(on_wait=[w], on_update=[])
                        newlist.append(nop)
                    ins.sync_info = mybir.SyncInfo(
                        on_wait=[waits[-1]], on_update=list(si.on_update or []))
                newlist.append(ins)
            blk.instructions = newlist
    return n


def build_nc():
    nc = bass.Bass()
    AL = mybir.AluOpType
    AX = mybir.AxisListType

    xt_d = nc.dram_tensor("xt", [E, BP], FP32, kind="ExternalInput")       # embed^T slice
    bias_d = nc.dram_tensor("biasv", [E, 1], FP32, kind="ExternalInput")
    biasr_d = nc.dram_tensor("biasr", [1, E], FP32, kind="ExternalInput")
    wth_d = nc.dram_tensor("wth", [E, F], FP32R, kind="ExternalInput")     # W^T hi (13-bit clean)
    whf8_d = nc.dram_tensor("whf8", [E, F], FP8, kind="ExternalInput")     # fp8(W^T hi)
    wlf8_d = nc.dram_tensor("wlf8", [E, F], FP8, kind="ExternalInput")     # fp8(W^T lo * 2^12)
    lk_d = nc.dram_tensor("lk", [F, E], FP16, kind="ExternalInput")        # lookup fp16
    id16_d = nc.dram_tensor("id16", [128, 128], FP16, kind="ExternalInput")

    out_d = nc.dram_tensor("out_embed", [BP, E], FP32, kind="ExternalOutput")
    tot_d = nc.dram_tensor("out_totals", [1, F], FP32, kind="ExternalOutput")
    flg_d = nc.dram_tensor("out_flags", [128, 3 * NBT], FP32, kind="ExternalOutput")

    with TileContext(nc) as tc:
        with tc.tile_pool(name="singles", bufs=1) as sing, \
             tc.tile_pool(name="dram", bufs=1, space="DRAM") as dpool:
            proj_d = dpool.tile([NBT * 128, F], FP32)

            id16 = sing.tile([128, 128], FP16)
            nc.gpsimd.dma_start(id16[:], id16_d[:])
            ones1 = sing.tile([1, 128], FP32)
            nc.vector.memset(ones1[:], 1.0)
            ones128 = sing.tile([128, 1], BF16)
            nc.vector.memset(ones128[:], 1.0)
            biasrow = sing.tile([1, E], FP32)
            nc.gpsimd.dma_start(biasrow[:], biasr_d[:])
            flags = sing.tile([128, 3 * NBT], FP32)   # per bt: [count, danger, tau]
            taus = sing.tile([128, NBT], FP32)

            # ---------- Phase A: encode + chunk-top8 + proj -> DRAM ----------
            with tc.tile_pool(name="phAB", bufs=1) as pab:
                m8s = pab.tile([128, NBT * 2048], FP32)   # per-bt chunk-top8s
                with tc.tile_pool(name="phA", bufs=1) as pa, \
                     tc.tile_pool(name="wstream", bufs=3) as ws, \
                     tc.tile_pool(name="chb", bufs=3) as chb, \
                     tc.tile_pool(name="psA", bufs=1, space="PSUM") as psA:
                    xh_all = pa.tile([128, NE * BP], FP32R)
                    xh8_all = pa.tile([128, NE * BP], FP8)
                    xl8_all = pa.tile([128, NE * BP], FP8)
                    for e in range(NE):
                        xa = pa.tile([128, BP], FP32, name=f"xa{e}", tag="xa", bufs=2)
                        nc.sync.dma_start(xa[:], xt_d[e * 128:(e + 1) * 128, :])
                        bcol = pa.tile([128, 1], FP32, name=f"bc{e}", tag="bc", bufs=2)
                        nc.sync.dma_start(bcol[:], bias_d[e * 128:(e + 1) * 128, :])
                        nc.vector.tensor_scalar(xa[:], xa[:], bcol[:], None,
                                                op0=AL.subtract)
                        xh = xh_all[:, e * BP:(e + 1) * BP]
                        nc.vector.tensor_copy(xh, xa[:])
                        xl = pa.tile([128, BP], FP32, name=f"xl{e}", tag="xl", bufs=2)
                        nc.vector.tensor_tensor(out=xl[:], in0=xa[:],
                                                in1=xh.bitcast(FP32),
                                                op=AL.subtract)
                        nc.vector.tensor_copy(xh8_all[:, e * BP:(e + 1) * BP],
                                              xh.bitcast(FP32))
                        nc.vector.tensor_scalar(xl8_all[:, e * BP:(e + 1) * BP],
                                                xl[:], 4096.0, None, op0=AL.mult)

                    NSC = NE // 2  # 8 superchunks of 256 e
                    xh8r = xh8_all.rearrange("p (e b) -> p e b", b=BP)
                    xl8r = xl8_all.rearrange("p (e b) -> p e b", b=BP)
                    for fc in range(NFC):
                        pss = [psA.tile([128, 512], FP32, name=f"psA{bt}_{fc}",
                                        tag=f"psA{bt}") for bt in range(NBT)]
                        psc = [psA.tile([128, 512], FP32, name=f"psC{bt}_{fc}",
                                        tag=f"psC{bt}") for bt in range(NBT)]
                        for e in range(NE):
                            wh = ws.tile([128, 512], FP32R, name=f"wh{fc}_{e}", tag="wh")
                            nc.sync.dma_start(wh[:], wth_d[e * 128:(e + 1) * 128,
                                                           fc * 512:(fc + 1) * 512])
                            for bt in range(NBT):
                                xh_b = xh_all[:, e * BP + bt * 128:e * BP + (bt + 1) * 128]
                                nc.tensor.matmul(pss[bt][:], xh_b, wh[:],
                                                 start=(e == 0), stop=(e == NE - 1))
                        for sc in range(NSC):
                            w8h = ws.tile([128, 1024], FP8, name=f"w8h{fc}_{sc}",
                                          tag="w8h")
                            nc.sync.dma_start(
                                w8h[:].rearrange("p (i n) -> p i n", i=2),
                                whf8_d[sc * 256:(sc + 1) * 256,
                                       fc * 512:(fc + 1) * 512].rearrange(
                                           "(i p) n -> p i n", i=2))
                            w8l = ws.tile([128, 1024], FP8, name=f"w8l{fc}_{sc}",
                                          tag="w8l")
                            nc.sync.dma_start(
                                w8l[:].rearrange("p (i n) -> p i n", i=2),
                                wlf8_d[sc * 256:(sc + 1) * 256,
                                       fc * 512:(fc + 1) * 512].rearrange(
                                           "(i p) n -> p i n", i=2))
                            for bt in range(NBT):
                                xl8_b = xl8r[:, 2 * sc:2 * sc + 2,
                                             bt * 128:(bt + 1) * 128]
                                xh8_b = xh8r[:, 2 * sc:2 * sc + 2,
                                             bt * 128:(bt + 1) * 128]
                                nc.tensor.matmul(
                                    psc[bt][:], xl8_b,
                                    w8h[:].rearrange("p (i n) -> p i n", i=2),
                                    start=(sc == 0), stop=False,
                                    perf_mode=mybir.MatmulPerfMode.DoubleRow)
                                nc.tensor.matmul(
                                    psc[bt][:], xh8_b,
                                    w8l[:].rearrange("p (i n) -> p i n", i=2),
                                    start=False, stop=(sc == NSC - 1),
                                    perf_mode=mybir.MatmulPerfMode.DoubleRow)
                        for bt in range(NBT):
                            cb = chb.tile([128, 512], FP32, name=f"cb{bt}_{fc}",
                                          tag=f"cb{bt}")
                            nc.scalar.copy(out=cb[:], in_=pss[bt][:])
                            nc.vector.scalar_tensor_tensor(
                                out=cb[:], in0=psc[bt][:], scalar=float(2.0 ** -12),
                                in1=cb[:], op0=AL.mult, op1=AL.add)
                            for s in range(4):
                                slot = m8s[:, bt * 2048 + fc * 32 + s * 8:
                                           bt * 2048 + fc * 32 + (s + 1) * 8]
                                nc.vector.max(out=slot, in_=cb[:, s * 128:(s + 1) * 128])
                            nc.sync.dma_start(
                                proj_d[bt * 128:(bt + 1) * 128,
                                       fc * 512:(fc + 1) * 512], cb[:])

                # ---------- Phase B: merge -> tau per bt ----------
                with tc.tile_pool(name="phB", bufs=1) as pb:
                    for bt in range(NBT):
                        m8 = m8s[:, bt * 2048:(bt + 1) * 2048]
                        eighth = pb.tile([128, 256], FP32, name=f"e8{bt}", tag="e8",
                                         bufs=2)
                        nc.vector.tensor_copy(
                            eighth[:], m8.rearrange("p (c k) -> p c k", k=8)[:, :, 7])
                        t8 = pb.tile([128, 8], FP32, name=f"t8{bt}", tag="t8", bufs=2)
                        for r in range(8):
                            nc.vector.max(out=t8[:], in_=m8)
                            if r < 7:
                                nc.vector.match_replace(out=m8, in_to_replace=t8[:],
                                                        in_values=m8, imm_value=-1e30)
                        nc.vector.tensor_copy(taus[:, bt:bt + 1], t8[:, 7:8])
                        dm = pb.tile([128, 256], FP32, name=f"dm{bt}", tag="dm", bufs=2)
                        nc.vector.tensor_scalar(dm[:], eighth[:], taus[:, bt:bt + 1],
                                                None, op0=AL.is_ge)
                        nc.vector.tensor_reduce(out=flags[:, 3 * bt + 1:3 * bt + 2],
                                                in_=dm[:], axis=AX.X, op=AL.add)
                        nc.vector.tensor_copy(flags[:, 3 * bt + 2:3 * bt + 3],
                                              taus[:, bt:bt + 1])

            # ---------- Phase D: mask + transpose + decode + totals ----------
            GW = DGRP * 128  # 1024 f per group
            with tc.tile_pool(name="phD", bufs=1) as pd_, \
                 tc.tile_pool(name="lkp", bufs=2) as lkp, \
                 tc.tile_pool(name="mtp", bufs=2) as mtp, \
                 tc.tile_pool(name="psD", bufs=2, space="PSUM") as psD:
                acc = pd_.tile([128, NBT * E], FP32)
                nc.vector.memset(acc[:], 0.0)
                counts = pd_.tile([128, NBT], FP32)
                nc.vector.memset(counts[:], 0.0)
                biasf = pd_.tile([128, E], FP32)
                for ec in range(NEC):
                    psb = psD.tile([128, 512], FP32, name=f"psb{ec}", tag="psacc")
                    nc.tensor.matmul(psb[:], ones1[:],
                                     biasrow[:, ec * 512:(ec + 1) * 512],
                                     start=True, stop=True)
                    nc.vector.tensor_copy(biasf[:, ec * 512:(ec + 1) * 512], psb[:])

                for g in range(NG):
                    f0 = g * GW
                    lks = []
                    for j in range(DGRP):
                        fb = g * DGRP + j
                        lk = lkp.tile([128, E], FP16, name=f"lk{j}_{g}", tag=f"lk{j}",
                                      bufs=1)
                        nc.sync.dma_start(lk[:], lk_d[fb * 128:(fb + 1) * 128, :])
                        lks.append(lk)
                    mts = {}
                    mbs = {}
                    for bt in range(NBT):
                        pj = pd_.tile([128, GW], FP32, name=f"pj{bt}_{g}",
                                      tag=f"pj{bt}", bufs=2)
                        nc.sync.dma_start(pj[:], proj_d[bt * 128:(bt + 1) * 128,
                                                        f0:f0 + GW])
                        mb = pd_.tile([128, GW], BF16, name=f"mbm{bt}_{g}",
                                      tag=f"mbm{bt}", bufs=2)
                        mbs[bt] = mb
                        nc.vector.tensor_scalar(mb[:], pj[:], taus[:, bt:bt + 1],
                                                None, op0=AL.is_ge)
                        m16 = pd_.tile([128, GW], FP16, name=f"m16{bt}_{g}",
                                       tag=f"m16{bt}", bufs=2)
                        nc.vector.tensor_tensor(out=m16[:], in0=pj[:], in1=mb[:],
                                                op=AL.mult)
                        ctmp = pd_.tile([128, 1], FP32, name=f"ct{bt}_{g}",
                                        tag=f"ct{bt}", bufs=2)
                        nc.vector.tensor_reduce(out=ctmp[:], in_=mb[:], axis=AX.X,
                                                op=AL.add)
                        nc.vector.tensor_tensor(out=counts[:, bt:bt + 1],
                                                in0=counts[:, bt:bt + 1],
                                                in1=ctmp[:], op=AL.add)
                        for j in range(DGRP):
                            pst = psD.tile([128, 128], FP16, name=f"pst{j}_{bt}_{g}",
                                           tag="pst")
                            nc.tensor.transpose(pst[:], m16[:, j * 128:(j + 1) * 128],
                                                id16[:])
                            mt = mtp.tile([128, 128], FP16, name=f"mt{j}_{bt}_{g}",
                                          tag=f"mt{j}_{bt}", bufs=1)
                            nc.vector.tensor_copy(mt[:], pst[:])
                            mts[(j, bt)] = mt
                    # totals: column-sums of mask via ones-matmul, chained over bt
                    for h in range(GW // 512):
                        pstot = psD.tile([1, 512], FP32, name=f"pt{h}_{g}", tag="ptot")
                        for bt in range(NBT):
                            nc.tensor.matmul(
                                pstot[:], ones128[:],
                                mbs[bt][:, h * 512:(h + 1) * 512],
                                start=(bt == 0), stop=(bt == NBT - 1))
                        tst = pd_.tile([1, 512], FP32, name=f"ts{h}_{g}",
                                       tag="tst", bufs=2)
                        nc.vector.tensor_copy(tst[:], pstot[:])
                        nc.sync.dma_start(
                            tot_d[:, f0 + h * 512:f0 + (h + 1) * 512], tst[:])
                    for bt in range(NBT):
                        for ec in range(NEC):
                            ps = psD.tile([128, 512], FP32, name=f"da{bt}_{ec}_{g}",
                                          tag="psacc")
                            for j in range(DGRP):
                                nc.tensor.matmul(ps[:], mts[(j, bt)][:],
                                                 lks[j][:, ec * 512:(ec + 1) * 512],
                                                 start=(j == 0), stop=(j == DGRP - 1))
                            a = acc[:, bt * E + ec * 512:bt * E + (ec + 1) * 512]
                            nc.vector.tensor_tensor(out=a, in0=a, in1=ps[:], op=AL.add)

                # ---------- Phase E: bias + normalize + outputs ----------

                for bt in range(NBT):
                    st = pd_.tile([128, E], FP32, name=f"st{bt}", tag="st", bufs=1)
                    nc.vector.tensor_tensor(out=st[:], in0=acc[:, bt * E:(bt + 1) * E],
                                            in1=biasf[:], op=AL.add)
                    sq = pd_.tile([128, E], FP32, name=f"sq{bt}", tag="sq", bufs=1)
                    nc.vector.tensor_tensor(out=sq[:], in0=st[:], in1=st[:], op=AL.mult)
                    ss = pd_.tile([128, 1], FP32, name=f"ss{bt}", tag="ss", bufs=2)
                    nc.vector.tensor_reduce(out=ss[:], in_=sq[:], axis=AX.X, op=AL.add)
                    nrm = pd_.tile([128, 1], FP32, name=f"nm{bt}", tag="nm", bufs=2)
                    nc.scalar.sqrt(out=nrm[:], in_=ss[:])
                    nc.vector.tensor_scalar_max(nrm[:], nrm[:], 1e-12)
                    rcp = pd_.tile([128, 1], FP32, name=f"rc{bt}", tag="rc", bufs=2)
                    nc.vector.reciprocal(out=rcp[:], in_=nrm[:])
                    nc.vector.tensor_scalar(st[:], st[:], rcp[:], None, op0=AL.mult)
                    nc.sync.dma_start(out_d[bt * 128:(bt + 1) * 128, :], st[:])
                    nc.vector.tensor_copy(flags[:, 3 * bt:3 * bt + 1],
                                          counts[:, bt:bt + 1])

                nc.sync.dma_start(flg_d[:], flags[:])

    return nc


def _drop_bits(v, drop=13):
    b = v.view(np.uint32).astype(np.uint64)
    r = ((b + (1 << (drop - 1))) >> drop << drop).astype(np.uint32)
    return r.view(np.float32)


_NC_CACHE = [None]


def kernel(embed, last_usage, bias, enc_weight, lookup):
    global LAST_EXEC_NS
    embed = np.ascontiguousarray(np.asarray(embed, dtype=np.float32))
    last_usage = np.asarray(last_usage, dtype=np.int32)
    bias = np.ascontiguousarray(np.asarray(bias, dtype=np.float32))
    enc_weight = np.ascontiguousarray(np.asarray(enc_weight, dtype=np.float32))
    lookup = np.ascontiguousarray(np.asarray(lookup, dtype=np.float32))

    import ml_dtypes
    wt = np.ascontiguousarray(enc_weight.T)            # [E, F]
    wth = _drop_bits(wt)
    wtl = (wt - wth).astype(np.float32)
    whf8 = wth.astype(ml_dtypes.float8_e4m3).view(np.uint8)
    wlf8 = (wtl * 4096.0).astype(ml_dtypes.float8_e4m3).view(np.uint8)
    lk16 = lookup.astype(np.float16)
    id16 = np.eye(128, dtype=np.float16)
    biasv = bias.reshape(E, 1).copy()
    biasr = bias.reshape(1, E).copy()

    in_maps = []
    for c in range(NCORES):
        xt = np.ascontiguousarray(embed[c * BP:(c + 1) * BP, :].T)
        in_maps.append({"xt": xt, "biasv": biasv, "biasr": biasr,
                        "wth": wth, "whf8": whf8, "wlf8": wlf8,
                        "lk": lk16, "id16": id16})

    if _NC_CACHE[0] is None:
        nc = build_nc()
        _split_multiwait(nc)
        _NC_CACHE[0] = nc
    nc = _NC_CACHE[0]

    res = run_bass_kernel_spmd(nc, in_maps, core_ids=list(range(NCORES)))
    LAST_EXEC_NS = res.exec_time_ns

    embed1 = np.concatenate([res.results[c]["out_embed"] for c in range(NCORES)],
                            axis=0)

    total = np.zeros(F, dtype=np.float64)
    for c in range(NCORES):
        total += res.results[c]["out_totals"].reshape(F).astype(np.float64)
    selected = total > 0.0
    new_last_usage = ((last_usage.astype(np.int64) + B) * (~selected)).astype(np.int32)

    for c in range(NCORES):
        fl = res.results[c]["out_flags"]
        for bt in range(NBT):
            cnt, dng, tau = fl[:, 3 * bt], fl[:, 3 * bt + 1], fl[:, 3 * bt + 2]
            if not np.all(cnt == 64.0):
                print(f"WARNING core{c} bt{bt}: counts != 64:",
                      np.unique(cnt[cnt != 64.0])[:8])
            if not np.all(dng == 0.0):
                print(f"WARNING core{c} bt{bt}: danger chunks present")
            if not np.all(tau > 0.0):
                print(f"WARNING core{c} bt{bt}: tau <= 0")

    return embed1, new_last_usage
